# revision 3
# baseline (speedup 1.0000x reference)
"""CQAttention Trainium2 kernel (8-core data parallel), v3.

Math (per example):
    S[i,j] = C@w_c [i] + Q@w_q [j] + (C*w_mul)@Q^T [i,j] + bias
    S1 = softmax_j(where(Qmask==0, -1e9, S))
    S2 = softmax_i(where(Cmask==0, -1e9, S))
    A  = S1 @ Q
    Bm = S1 @ S2^T @ C
    out = concat([C, A, C*A, C*Bm], axis=-1)

Key identities (same math as v2):
  - softmax shift-invariance: `bias` drops out; per-row offsets drop out of
    S1; per-column offsets drop out of S2.
  - With Qm'[d,j] = w_mul[d]*Q[j,d] + w_c[d] (host-packed) and
    bias1[j] = (Q@w_q)[j] + qneg[j]:
        E^T[j,i] = exp(Qm'^T@C^T + bias1[j])    one matmul per example.
  - T' = rownorm(S2^T@C) is invariant to per-j scaling, so the S2 path
    reuses E^T via an XBAR DMA transpose (eu) and the host-packed masked
    row-major C (CMB).
  - abm per Lc-tile: [A_raw | Bm_raw | r] = E_tile^T.T @ [Q | T' | 1];
    host divides by r and forms C*A / C*Bm during f32 assembly.

v3 restructuring (v2 measured 62.5us; phases serialized, PE went cold):
  - Per-example software pipeline: score-mm -> exp -> transpose -> traw ->
    abm -> drain -> store, emitted with a ~5-example lead on the score
    phase so every engine stream stays dense.
  - e1 exp is ONE [128,1024] activation per example (2-bank PSUM read).
  - PSUM repacked to exactly 8 banks: e1 [128,1024]x1, traw [128,132]x2,
    abm [128,1024]x2, warmup [128,512]x1.
  - Drains split scalar/vector to balance (~2.1us/engine/example incl exp).
  - OUT stores issued from the (otherwise idle) gpsimd SWDGE ring so the
    sync ring only carries input loads + the 4 XBAR transposes.
  - Dummy warmup matmuls interleaved into the ramp keep HAM at K=8/8.
"""

import os
import sys
from contextlib import ExitStack

import ml_dtypes
import numpy as np

for _p in ("/opt/trn_rl_repo", "/root/.axon_site/_ro/trn_rl_repo"):
    if os.path.isdir(_p) and _p not in sys.path:
        sys.path.append(_p)

import concourse.bass as bass
import concourse.tile as tile
from concourse import bacc, mybir
from concourse.bass import ds, ts
from concourse.bass_utils import run_bass_kernel_spmd

F32 = mybir.dt.float32
FP16 = mybir.dt.float16
BF16 = mybir.dt.bfloat16
AF = mybir.ActivationFunctionType
ALU = mybir.AluOpType

N_CORES = 8
B, LC, LQ, D = 64, 1024, 128, 128
B_LOC = B // N_CORES  # 8 examples per core
NT = LC // 128  # 8 Lc tiles of 128


def _build_graph():
    nc = bacc.Bacc("TRN2", target_bir_lowering=False, debug=False)

    # pair-packed loads: [pair, p, e-in-pair, cols]
    CT = nc.dram_tensor("CT", [B_LOC // 2, D, 2, LC], FP16, kind="ExternalInput").ap()
    CMB = nc.dram_tensor(
        "CMB", [B_LOC // 2, 128, 2, NT * 130], BF16, kind="ExternalInput"
    ).ap()
    QM = nc.dram_tensor("QM", [D, B_LOC * LQ], FP16, kind="ExternalInput").ap()
    QS = nc.dram_tensor("QS", [LQ, B_LOC * D], BF16, kind="ExternalInput").ap()
    B1 = nc.dram_tensor("B1", [LQ, B_LOC], F32, kind="ExternalInput").ap()
    # per-tile raw rows: OUT[e][m, t*257+n] = [A_raw | Bm_raw | r][128t+m, n]
    OUT = nc.dram_tensor("OUT", [B_LOC, 128, NT * 257], BF16, kind="ExternalOutput").ap()

    with tile.TileContext(nc) as tc:
        with ExitStack() as ctx:
            ep = ctx.enter_context

            const = ep(tc.tile_pool(name="const", bufs=1))
            p_ct = ep(tc.tile_pool(name="ct", bufs=B_LOC // 2))
            p_cxb = ep(tc.tile_pool(name="cxb", bufs=B_LOC // 2))
            p_eq = ep(tc.tile_pool(name="eq", bufs=4))
            p_eu = ep(tc.tile_pool(name="eu", bufs=3))
            p_rhs = ep(tc.tile_pool(name="rhs", bufs=B_LOC))
            p_stg = ep(tc.tile_pool(name="stg", bufs=3))
            p_small = ep(tc.tile_pool(name="small", bufs=24))

            pp_e1 = ep(tc.tile_pool(name="pp_e1", bufs=1, space="PSUM"))
            pp_traw = ep(tc.tile_pool(name="pp_traw", bufs=1, space="PSUM"))
            pp_abm = ep(tc.tile_pool(name="pp_abm", bufs=2, space="PSUM"))
            pp_warm = ep(tc.tile_pool(name="pp_warm", bufs=1, space="PSUM"))

            # ---- input loads ----
            # scalar hwdge ring: small tensors needed first
            qm_all = const.tile([D, B_LOC * LQ], FP16)
            nc.scalar.dma_start(qm_all, QM)
            b1_sb = const.tile([LQ, B_LOC], F32)
            nc.scalar.dma_start(b1_sb, B1)
            qs_all = const.tile([LQ, B_LOC, D], BF16)
            nc.scalar.dma_start(qs_all, QS.rearrange("p (e d) -> p e d", d=D))
            # sync hwdge ring: bulk pair loads (CT/CMB interleaved so early
            # examples arrive first), then the 4 XBAR transposes
            cts, cxbs, eqs, eus, rhss, stgs = {}, {}, {}, {}, {}, {}
            for pr in range(B_LOC // 2):
                ct = p_ct.tile([128, 2, LC], FP16, tag="ct", name=f"ct_{pr}")
                nc.sync.dma_start(ct, CT[pr])
                cts[2 * pr] = ct[:, 0, :]
                cts[2 * pr + 1] = ct[:, 1, :]
                cxb = p_cxb.tile([128, 2, NT * 130], BF16, tag="cxb", name=f"cxb_{pr}")
                nc.sync.dma_start(cxb, CMB[pr])
                cxbs[2 * pr] = cxb[:, 0, :]
                cxbs[2 * pr + 1] = cxb[:, 1, :]

            # PE warmup: dense dummy matmuls with no data deps. Emitted
            # interleaved into the ramp (see emit_warm calls) so the PE has
            # filler while the e1/exp/transpose pipeline fills, keeping HAM
            # at K=8/8 from ~3.5us on.
            warm_w = const.tile([128, 512], BF16)
            nc.vector.memset(warm_w, 1.0)

            def emit_warm(n):
                for _ in range(n):
                    warm_ps = pp_warm.tile([128, 512], F32, tag="warm")
                    nc.tensor.matmul(
                        warm_ps[:, 0:256], lhsT=warm_w[:, 0:128], rhs=warm_w[:, 0:256]
                    )

            # rhs tiles [Q | T' | 1] on gpsimd (idle engine; qs arrives early)
            for e in range(B_LOC):
                rhs = p_rhs.tile([128, 260], BF16, tag="rhs", name=f"rhs_{e}")
                nc.gpsimd.tensor_copy(rhs[:, 0:128], qs_all[:, e, :])
                nc.gpsimd.memset(rhs[:, 256:257], 1.0)
                rhss[e] = rhs

            eq_batches = {}

            def emit_e1(e):
                # scores + exp for one example; eq batched 2 examples per
                # tile so one XBAR transpose covers both
                if e % 2 == 0:
                    eq_batches[e // 2] = p_eq.tile(
                        [128, 2, LC], BF16, tag="eq", name=f"eqb_{e // 2}"
                    )
                eq = eq_batches[e // 2][:, e % 2, :]
                ps = pp_e1.tile([128, 1024], F32, tag="pe1", name=f"e1ps_{e}")
                for h in range(2):
                    nc.tensor.matmul(
                        ps[:, ts(h, 512)],
                        lhsT=qm_all[:, ts(e, LQ)],
                        rhs=cts[e][:, ts(h, 512)],
                    )
                # one [128,1024] activation: exp(S^T + b1) -> bf16
                nc.scalar.activation(
                    eq, ps, func=AF.Exp, bias=b1_sb[:, e : e + 1], scale=1.0
                )
                eqs[e] = eq

            def emit_xpose(b):
                # eu[p, u, j] = E^T[j, 128u+p] over the 2-example batch
                eu = p_eu.tile([128, 2 * NT, 128], BF16, tag="eu", name=f"eub_{b}")
                nc.sync.dma_start_transpose(
                    eu, eq_batches[b].rearrange("p a x -> p (a x)")
                )
                for ee in (2 * b, 2 * b + 1):
                    eus[ee] = eu[:, NT * (ee % 2) : NT * (ee % 2) + NT, :]

            def emit_traw(e):
                traw_ps = pp_traw.tile([128, 132], F32, tag="ptraw", name=f"traw_{e}")
                for t in range(NT):
                    nc.tensor.matmul(
                        traw_ps[:, 0:129],
                        lhsT=eus[e][:, t, :],
                        rhs=cxbs[e][:, ds(130 * t, 129)],
                        start=(t == 0),
                        stop=(t == NT - 1),
                    )
                c_sb = p_small.tile([128, 1], F32, tag="small", name=f"c_{e}")
                nc.vector.tensor_scalar_add(c_sb, traw_ps[:, 128:129], 1e-30)
                cinv = p_small.tile([128, 1], F32, tag="small", name=f"cinv_{e}")
                nc.vector.reciprocal(cinv, c_sb)
                nc.vector.tensor_scalar_mul(
                    rhss[e][:, 128:256], traw_ps[:, 0:128], cinv
                )

            def emit_abm(e):
                stg = p_stg.tile([128, NT, 257], BF16, tag="stg", name=f"stg_{e}")
                for pr in range(NT // 2):
                    ps = pp_abm.tile([128, 1024], F32, tag="pabm", name=f"abm_{e}_{pr}")
                    for k in range(2):
                        nc.tensor.matmul(
                            ps[:, ds(512 * k, 257)],
                            lhsT=eqs[e][:, ts(2 * pr + k, 128)],
                            rhs=rhss[e][:, 0:257],
                        )
                    src = bass.AP(
                        tensor=ps.tensor,
                        offset=ps.offset,
                        ap=[ps.ap[0], [512, 2], [1, 257]],
                    )
                    dst = stg[:, 2 * pr : 2 * pr + 2, :]
                    # drain split keeps scalar (which also runs the exps) and
                    # vector near-equally loaded across an example pair
                    on_scalar = (pr == 0) or (pr == 1 and e % 2 == 0)
                    if on_scalar:
                        nc.scalar.copy(dst, src)
                    else:
                        nc.vector.tensor_copy(dst, src)
                stgs[e] = stg

            def emit_store(e):
                # swdge ring on gpsimd: keeps stores off the sync ring where
                # they would queue behind the XBAR transposes
                nc.gpsimd.dma_start(
                    OUT[e].rearrange("p (t x) -> p t x", x=257), stgs[e]
                )

            # ---- software pipeline ----
            # score phase leads by ~5 examples; steady-state rounds run
            # traw(e)+abm(e) on the PE while scalar exps example e+4/e+5 and
            # the XBAR transposes example pair (e+2)/2.
            emit_warm(6)
            emit_e1(0)
            emit_warm(4)
            emit_e1(1)
            emit_xpose(0)
            emit_warm(4)
            emit_e1(2)
            emit_warm(4)
            emit_e1(3)
            emit_xpose(1)
            emit_warm(4)
            emit_e1(4)
            emit_e1(5)
            emit_xpose(2)

            emit_traw(0)
            emit_abm(0)
            emit_store(0)
            emit_traw(1)
            emit_abm(1)
            emit_store(1)
            emit_traw(2)
            emit_abm(2)
            emit_store(2)
            emit_e1(6)
            emit_e1(7)
            emit_xpose(3)
            emit_traw(3)
            emit_abm(3)
            emit_store(3)
            for e in range(4, B_LOC):
                emit_traw(e)
                emit_abm(e)
                emit_store(e)

    nc.compile()
    return nc


_GRAPH = None


def _graph():
    global _GRAPH
    if _GRAPH is None:
        _GRAPH = _build_graph()
    return _GRAPH


def make_in_maps(C, Q, Cmask, Qmask, w_c, w_q, w_mul):
    """Shard full inputs into per-core input maps (host-side layout prep)."""
    C = np.asarray(C, dtype=np.float32)
    Q = np.asarray(Q, dtype=np.float32)
    wmul_r = np.asarray(w_mul, dtype=np.float32).reshape(D)
    wc_r = np.asarray(w_c, dtype=np.float32).reshape(D)
    wq_r = np.asarray(w_q, dtype=np.float32).reshape(D)
    in_maps = []
    for i in range(N_CORES):
        sl = slice(i * B_LOC, (i + 1) * B_LOC)
        Ci = C[sl]
        Qi = Q[sl]
        cmi = np.asarray(Cmask[sl], dtype=np.float32)  # [8, 1024]
        qneg = (np.asarray(Qmask[sl], dtype=np.float32) - 1.0) * 1e9  # [8, 128]
        # Qm'[e][d, j] = wmul[d]*Q[e,j,d] + wc[d], packed [128, 8*128] fp16
        qm = Qi.transpose(0, 2, 1) * wmul_r[None, :, None] + wc_r[None, :, None]
        qm = np.ascontiguousarray(
            qm.astype(np.float16).transpose(1, 0, 2).reshape(D, B_LOC * LQ)
        )
        # Q row-major, [j, e*128+d] bf16
        qs = np.ascontiguousarray(
            Qi.astype(ml_dtypes.bfloat16).transpose(1, 0, 2).reshape(LQ, B_LOC * D)
        )
        # C^T pair-packed: [pair, d, e-in-pair, i]
        ct = np.ascontiguousarray(
            Ci.transpose(0, 2, 1)
            .astype(np.float16)
            .reshape(B_LOC // 2, 2, D, LC)
            .transpose(0, 2, 1, 3)
        )
        # p-major packed masked C: [e, p, t*130+x] = (cm*C)[128t+p, x] | cm | 0
        cmb = np.zeros((B_LOC, LC, 130), dtype=ml_dtypes.bfloat16)
        cmb[:, :, 0:128] = (Ci * cmi[:, :, None]).astype(ml_dtypes.bfloat16)
        cmb[:, :, 128] = cmi.astype(ml_dtypes.bfloat16)
        cmb = (
            cmb.reshape(B_LOC, NT, 128, 130)
            .transpose(0, 2, 1, 3)
            .reshape(B_LOC // 2, 2, 128, NT * 130)
            .transpose(0, 2, 1, 3)
        )
        cmb = np.ascontiguousarray(cmb)
        # bias1[j, e] = (Q[e] @ wq)[j] + qneg[e, j]
        s1 = Qi @ wq_r  # [8, 128]
        b1 = np.ascontiguousarray((s1 + qneg).T.astype(np.float32))
        in_maps.append(
            {
                "CT": ct,
                "QM": qm,
                "QS": qs,
                "CMB": cmb,
                "B1": b1,
            }
        )
    return in_maps


def assemble(results, C):
    """Gather per-core raw device outputs + input C into the full f32 output."""
    C = np.asarray(C, dtype=np.float32)
    out = np.empty((B, LC, 4 * D), dtype=np.float32)
    out[:, :, 0:D] = C
    for i in range(N_CORES):
        sl = slice(i * B_LOC, (i + 1) * B_LOC)
        o = np.asarray(results[i]["OUT"]).reshape(B_LOC, 128, NT, 257)
        o = o.astype(np.float32)
        a_raw = o[..., 0:128].transpose(0, 2, 1, 3).reshape(B_LOC, LC, D)
        b_raw = o[..., 128:256].transpose(0, 2, 1, 3).reshape(B_LOC, LC, D)
        r = o[..., 256].transpose(0, 2, 1).reshape(B_LOC, LC, 1)
        r = np.maximum(r, 1e-30)
        A = a_raw / r
        Bm = b_raw / r
        Ci = C[sl]
        out[sl, :, D : 2 * D] = A
        out[sl, :, 2 * D : 3 * D] = Ci * A
        out[sl, :, 3 * D : 4 * D] = Ci * Bm
    return out


def kernel(C, Q, Cmask, Qmask, w_c, w_q, w_mul, bias=None, **_ignored):
    # `bias` is mathematically a no-op: it shifts every score equally and
    # softmax is shift-invariant, so the output does not depend on it.
    nc = _graph()
    in_maps = make_in_maps(C, Q, Cmask, Qmask, w_c, w_q, w_mul)
    res = run_bass_kernel_spmd(nc, in_maps, core_ids=list(range(N_CORES)))
    return assemble(res.results, C)


# revision 8
# speedup vs baseline: 1.0710x; 1.0710x over previous
"""CQAttention Trainium2 kernel (8-core data parallel), v3.

Math (per example):
    S[i,j] = C@w_c [i] + Q@w_q [j] + (C*w_mul)@Q^T [i,j] + bias
    S1 = softmax_j(where(Qmask==0, -1e9, S))
    S2 = softmax_i(where(Cmask==0, -1e9, S))
    A  = S1 @ Q
    Bm = S1 @ S2^T @ C
    out = concat([C, A, C*A, C*Bm], axis=-1)

Key identities (same math as v2):
  - softmax shift-invariance: `bias` drops out; per-row offsets drop out of
    S1; per-column offsets drop out of S2.
  - With Qm'[d,j] = w_mul[d]*Q[j,d] + w_c[d] (host-packed) and
    bias1[j] = (Q@w_q)[j] + qneg[j]:
        E^T[j,i] = exp(Qm'^T@C^T + bias1[j])    one matmul per example.
  - T' = rownorm(S2^T@C) is invariant to per-j scaling, so the S2 path
    reuses E^T via an XBAR DMA transpose (eu) and the host-packed masked
    row-major C (CMB).
  - abm per Lc-tile: [A_raw | Bm_raw | r] = E_tile^T.T @ [Q | T' | 1];
    host divides by r and forms C*A / C*Bm during f32 assembly.

v3 restructuring (v2 measured 62.5us; phases serialized, PE went cold):
  - Per-example software pipeline: score-mm -> exp -> transpose -> traw ->
    abm -> drain -> store, emitted with a ~5-example lead on the score
    phase so every engine stream stays dense.
  - e1 exp is ONE [128,1024] activation per example (2-bank PSUM read).
  - PSUM repacked to exactly 8 banks: e1 [128,1024]x1, traw [128,132]x2,
    abm [128,1024]x2, warmup [128,512]x1.
  - Drains split scalar/vector to balance (~2.1us/engine/example incl exp).
  - OUT stores issued from the (otherwise idle) gpsimd SWDGE ring so the
    sync ring only carries input loads + the 4 XBAR transposes.
  - Dummy warmup matmuls interleaved into the ramp keep HAM at K=8/8.
"""

import os
import sys
from contextlib import ExitStack

import ml_dtypes
import numpy as np

for _p in ("/opt/trn_rl_repo", "/root/.axon_site/_ro/trn_rl_repo"):
    if os.path.isdir(_p) and _p not in sys.path:
        sys.path.append(_p)

import concourse.bass as bass
import concourse.tile as tile
from concourse import bacc, mybir
from concourse.bass import ds, ts
from concourse.bass_utils import run_bass_kernel_spmd

F32 = mybir.dt.float32
FP16 = mybir.dt.float16
BF16 = mybir.dt.bfloat16
AF = mybir.ActivationFunctionType
ALU = mybir.AluOpType

N_CORES = 8
B, LC, LQ, D = 64, 1024, 128, 128
B_LOC = B // N_CORES  # 8 examples per core
NT = LC // 128  # 8 Lc tiles of 128


def _build_graph():
    nc = bacc.Bacc("TRN2", target_bir_lowering=False, debug=False)

    # pair-packed loads: [pair, p, e-in-pair, cols]
    CT = nc.dram_tensor("CT", [B_LOC // 2, D, 2, LC], FP16, kind="ExternalInput").ap()
    CMB = nc.dram_tensor(
        "CMB", [B_LOC // 2, 128, 2, NT * 130], BF16, kind="ExternalInput"
    ).ap()
    QM = nc.dram_tensor("QM", [D, B_LOC * LQ], FP16, kind="ExternalInput").ap()
    QS = nc.dram_tensor("QS", [LQ, B_LOC * D], BF16, kind="ExternalInput").ap()
    B1 = nc.dram_tensor("B1", [LQ, B_LOC], F32, kind="ExternalInput").ap()
    # per-tile raw rows: OUT[e][m, t*257+n] = [A_raw | Bm_raw | r][128t+m, n]
    OUT = nc.dram_tensor("OUT", [B_LOC, 128, NT * 257], BF16, kind="ExternalOutput").ap()

    with tile.TileContext(nc) as tc:
        with ExitStack() as ctx:
            ep = ctx.enter_context

            const = ep(tc.tile_pool(name="const", bufs=1))
            p_ct = ep(tc.tile_pool(name="ct", bufs=B_LOC // 2))
            p_cxb = ep(tc.tile_pool(name="cxb", bufs=B_LOC // 2))
            p_eq = ep(tc.tile_pool(name="eq", bufs=4))
            p_eu = ep(tc.tile_pool(name="eu", bufs=3))
            p_rhs = ep(tc.tile_pool(name="rhs", bufs=B_LOC))
            p_stg = ep(tc.tile_pool(name="stg", bufs=4))
            p_small = ep(tc.tile_pool(name="small", bufs=24))

            # PSUM: e1 3x[128,512] (3 banks) + traw 1 + abm 2x[128,1024] (4) = 8
            pp_e1 = ep(tc.tile_pool(name="pp_e1", bufs=3, space="PSUM"))
            pp_traw = ep(tc.tile_pool(name="pp_traw", bufs=1, space="PSUM"))
            pp_abm = ep(tc.tile_pool(name="pp_abm", bufs=2, space="PSUM"))

            # ---- input loads ----
            # scalar hwdge ring: small tensors needed first
            qm_all = const.tile([D, B_LOC * LQ], FP16)
            nc.scalar.dma_start(qm_all, QM)
            b1_sb = const.tile([LQ, B_LOC], F32)
            nc.scalar.dma_start(b1_sb, B1)
            qs_all = const.tile([LQ, B_LOC, D], BF16)
            nc.scalar.dma_start(qs_all, QS.rearrange("p (e d) -> p e d", d=D))
            # sync hwdge ring: bulk pair loads (CT/CMB interleaved so early
            # examples arrive first), then the 4 XBAR transposes
            cts, cxbs, eqs, eus, rhss, stgs = {}, {}, {}, {}, {}, {}
            for pr in range(B_LOC // 2):
                ct = p_ct.tile([128, 2, LC], FP16, tag="ct", name=f"ct_{pr}")
                nc.sync.dma_start(ct, CT[pr])
                cts[2 * pr] = ct[:, 0, :]
                cts[2 * pr + 1] = ct[:, 1, :]
                cxb = p_cxb.tile([128, 2, NT * 130], BF16, tag="cxb", name=f"cxb_{pr}")
                nc.sync.dma_start(cxb, CMB[pr])
                cxbs[2 * pr] = cxb[:, 0, :]
                cxbs[2 * pr + 1] = cxb[:, 1, :]

            # PE warmup: dense dummy matmuls with no data deps (they borrow
            # the abm PSUM pool, which is idle during the ramp). Emitted
            # interleaved into the ramp (see emit_warm calls) so the PE has
            # filler while the e1/exp/transpose pipeline fills, keeping HAM
            # at K=8/8 from ~3.5us on.
            warm_w = const.tile([128, 512], BF16)
            nc.vector.memset(warm_w, 1.0)

            def emit_warm(n):
                for _ in range(n):
                    warm_ps = pp_abm.tile([128, 512], F32, tag="pabm")
                    nc.tensor.matmul(
                        warm_ps[:, 0:256], lhsT=warm_w[:, 0:128], rhs=warm_w[:, 0:256]
                    )

            # rhs tiles [Q | T' | 1] on gpsimd (idle engine; qs arrives early)
            for e in range(B_LOC):
                rhs = p_rhs.tile([128, 260], BF16, tag="rhs", name=f"rhs_{e}")
                nc.gpsimd.tensor_copy(rhs[:, 0:128], qs_all[:, e, :])
                nc.gpsimd.memset(rhs[:, 256:257], 1.0)
                rhss[e] = rhs

            eq_batches = {}

            def emit_e1(e):
                # scores + exp for one example; eq batched 2 examples per
                # tile so one XBAR transpose covers both
                if e % 2 == 0:
                    eq_batches[e // 2] = p_eq.tile(
                        [128, 2, LC], BF16, tag="eq", name=f"eqb_{e // 2}"
                    )
                eq = eq_batches[e // 2][:, e % 2, :]
                # two 1-bank PSUM tiles from a 3-buf pool: M1(e+1) never
                # waits more than one exp-drain back (the v3 bufs=1 layout
                # hard-serialized the whole e1 chain at ~2.2us/example)
                for h in range(2):
                    ps = pp_e1.tile([128, 512], F32, tag="pe1", name=f"e1ps_{e}_{h}")
                    nc.tensor.matmul(
                        ps,
                        lhsT=qm_all[:, ts(e, LQ)],
                        rhs=cts[e][:, ts(h, 512)],
                    )
                    nc.scalar.activation(
                        eq[:, ts(h, 512)],
                        ps,
                        func=AF.Exp,
                        bias=b1_sb[:, e : e + 1],
                        scale=1.0,
                    )
                eqs[e] = eq

            def emit_xpose(b):
                # eu[p, u, j] = E^T[j, 128u+p] over the 2-example batch
                eu = p_eu.tile([128, 2 * NT, 128], BF16, tag="eu", name=f"eub_{b}")
                nc.sync.dma_start_transpose(
                    eu, eq_batches[b].rearrange("p a x -> p (a x)")
                )
                for ee in (2 * b, 2 * b + 1):
                    eus[ee] = eu[:, NT * (ee % 2) : NT * (ee % 2) + NT, :]

            def emit_traw(e):
                traw_ps = pp_traw.tile([128, 132], F32, tag="ptraw", name=f"traw_{e}")
                for t in range(NT):
                    nc.tensor.matmul(
                        traw_ps[:, 0:129],
                        lhsT=eus[e][:, t, :],
                        rhs=cxbs[e][:, ds(130 * t, 129)],
                        start=(t == 0),
                        stop=(t == NT - 1),
                    )
                c_sb = p_small.tile([128, 1], F32, tag="small", name=f"c_{e}")
                nc.vector.tensor_scalar_add(c_sb, traw_ps[:, 128:129], 1e-30)
                cinv = p_small.tile([128, 1], F32, tag="small", name=f"cinv_{e}")
                nc.vector.reciprocal(cinv, c_sb)
                nc.vector.tensor_scalar_mul(
                    rhss[e][:, 128:256], traw_ps[:, 0:128], cinv
                )

            def emit_abm(e):
                stg = p_stg.tile([128, NT, 257], BF16, tag="stg", name=f"stg_{e}")
                for pr in range(NT // 2):
                    ps = pp_abm.tile([128, 1024], F32, tag="pabm", name=f"abm_{e}_{pr}")
                    for k in range(2):
                        nc.tensor.matmul(
                            ps[:, ds(512 * k, 257)],
                            lhsT=eqs[e][:, ts(2 * pr + k, 128)],
                            rhs=rhss[e][:, 0:257],
                        )
                    src = bass.AP(
                        tensor=ps.tensor,
                        offset=ps.offset,
                        ap=[ps.ap[0], [512, 2], [1, 257]],
                    )
                    dst = stg[:, 2 * pr : 2 * pr + 2, :]
                    # pr0/pr1 drains gate the pr2/pr3 matmuls (2-buf PSUM
                    # rotation), so they go on vector where they issue right
                    # after V(e); scalar (busy with exps) takes only pr2
                    if pr == 2:
                        nc.scalar.copy(dst, src)
                    else:
                        nc.vector.tensor_copy(dst, src)
                stgs[e] = stg

            def emit_store(e):
                # swdge ring on gpsimd: keeps stores off the sync ring where
                # they would queue behind the XBAR transposes
                nc.gpsimd.dma_start(
                    OUT[e].rearrange("p (t x) -> p t x", x=257), stgs[e]
                )

            # ---- software pipeline ----
            # score phase leads by ~5 examples; steady-state rounds run
            # traw(e)+abm(e) on the PE while scalar exps example e+4/e+5 and
            # the XBAR transposes example pair (e+2)/2.
            emit_warm(6)
            emit_e1(0)
            emit_warm(4)
            emit_e1(1)
            emit_xpose(0)
            emit_warm(4)
            emit_e1(2)
            emit_warm(4)
            emit_e1(3)
            emit_xpose(1)
            emit_warm(4)
            emit_e1(4)
            emit_e1(5)
            emit_xpose(2)

            emit_traw(0)
            emit_abm(0)
            emit_store(0)
            emit_traw(1)
            emit_abm(1)
            emit_store(1)
            emit_traw(2)
            emit_abm(2)
            emit_store(2)
            emit_e1(6)
            emit_e1(7)
            emit_xpose(3)
            emit_traw(3)
            emit_abm(3)
            emit_store(3)
            for e in range(4, B_LOC):
                emit_traw(e)
                emit_abm(e)
                emit_store(e)

    nc.compile()
    return nc


_GRAPH = None


def _graph():
    global _GRAPH
    if _GRAPH is None:
        _GRAPH = _build_graph()
    return _GRAPH


def make_in_maps(C, Q, Cmask, Qmask, w_c, w_q, w_mul):
    """Shard full inputs into per-core input maps (host-side layout prep)."""
    C = np.asarray(C, dtype=np.float32)
    Q = np.asarray(Q, dtype=np.float32)
    wmul_r = np.asarray(w_mul, dtype=np.float32).reshape(D)
    wc_r = np.asarray(w_c, dtype=np.float32).reshape(D)
    wq_r = np.asarray(w_q, dtype=np.float32).reshape(D)
    in_maps = []
    for i in range(N_CORES):
        sl = slice(i * B_LOC, (i + 1) * B_LOC)
        Ci = C[sl]
        Qi = Q[sl]
        cmi = np.asarray(Cmask[sl], dtype=np.float32)  # [8, 1024]
        qneg = (np.asarray(Qmask[sl], dtype=np.float32) - 1.0) * 1e9  # [8, 128]
        # Qm'[e][d, j] = wmul[d]*Q[e,j,d] + wc[d], packed [128, 8*128] fp16
        qm = Qi.transpose(0, 2, 1) * wmul_r[None, :, None] + wc_r[None, :, None]
        qm = np.ascontiguousarray(
            qm.astype(np.float16).transpose(1, 0, 2).reshape(D, B_LOC * LQ)
        )
        # Q row-major, [j, e*128+d] bf16
        qs = np.ascontiguousarray(
            Qi.astype(ml_dtypes.bfloat16).transpose(1, 0, 2).reshape(LQ, B_LOC * D)
        )
        # C^T pair-packed: [pair, d, e-in-pair, i]
        ct = np.ascontiguousarray(
            Ci.transpose(0, 2, 1)
            .astype(np.float16)
            .reshape(B_LOC // 2, 2, D, LC)
            .transpose(0, 2, 1, 3)
        )
        # p-major packed masked C: [e, p, t*130+x] = (cm*C)[128t+p, x] | cm | 0
        cmb = np.zeros((B_LOC, LC, 130), dtype=ml_dtypes.bfloat16)
        cmb[:, :, 0:128] = (Ci * cmi[:, :, None]).astype(ml_dtypes.bfloat16)
        cmb[:, :, 128] = cmi.astype(ml_dtypes.bfloat16)
        cmb = (
            cmb.reshape(B_LOC, NT, 128, 130)
            .transpose(0, 2, 1, 3)
            .reshape(B_LOC // 2, 2, 128, NT * 130)
            .transpose(0, 2, 1, 3)
        )
        cmb = np.ascontiguousarray(cmb)
        # bias1[j, e] = (Q[e] @ wq)[j] + qneg[e, j]
        s1 = Qi @ wq_r  # [8, 128]
        b1 = np.ascontiguousarray((s1 + qneg).T.astype(np.float32))
        in_maps.append(
            {
                "CT": ct,
                "QM": qm,
                "QS": qs,
                "CMB": cmb,
                "B1": b1,
            }
        )
    return in_maps


def assemble(results, C):
    """Gather per-core raw device outputs + input C into the full f32 output."""
    C = np.asarray(C, dtype=np.float32)
    out = np.empty((B, LC, 4 * D), dtype=np.float32)
    out[:, :, 0:D] = C
    for i in range(N_CORES):
        sl = slice(i * B_LOC, (i + 1) * B_LOC)
        o = np.asarray(results[i]["OUT"]).reshape(B_LOC, 128, NT, 257)
        o = o.astype(np.float32)
        a_raw = o[..., 0:128].transpose(0, 2, 1, 3).reshape(B_LOC, LC, D)
        b_raw = o[..., 128:256].transpose(0, 2, 1, 3).reshape(B_LOC, LC, D)
        r = o[..., 256].transpose(0, 2, 1).reshape(B_LOC, LC, 1)
        r = np.maximum(r, 1e-30)
        A = a_raw / r
        Bm = b_raw / r
        Ci = C[sl]
        out[sl, :, D : 2 * D] = A
        out[sl, :, 2 * D : 3 * D] = Ci * A
        out[sl, :, 3 * D : 4 * D] = Ci * Bm
    return out


def kernel(C, Q, Cmask, Qmask, w_c, w_q, w_mul, bias=None, **_ignored):
    # `bias` is mathematically a no-op: it shifts every score equally and
    # softmax is shift-invariant, so the output does not depend on it.
    nc = _graph()
    in_maps = make_in_maps(C, Q, Cmask, Qmask, w_c, w_q, w_mul)
    res = run_bass_kernel_spmd(nc, in_maps, core_ids=list(range(N_CORES)))
    return assemble(res.results, C)
